# revision 1
# baseline (speedup 1.0000x reference)
"""BondReactivityPredictor Trainium2 kernel.

Sharding: edges (E=400000) split data-parallel across 8 NeuronCores
(50000/core, padded to 51200 = 25 groups x 2048). node_embedding (+
atom logits, concatenated host-side into one gather table) and all MLP
weights are replicated on every core. Each core gathers node rows for
its edge shard with multi-row indirect DMAs and runs the edge MLP in a
feature-major layout (features on SBUF partitions, edges on the free
dim, 512 edges per matmul).
"""

import os
import sys

import numpy as np

for _p in ("/opt/trn_rl_repo", "/root/.axon_site/_ro/trn_rl_repo"):
    if os.path.isdir(_p) and _p not in sys.path:
        sys.path.insert(0, _p)

import concourse.bass as bass
import concourse.bacc as bacc
import concourse.mybir as mybir
import concourse.tile as tile
from concourse import library_config
from concourse.masks import make_identity

F32 = mybir.dt.float32
F32R = mybir.dt.float32r
I32 = mybir.dt.int32
I16 = mybir.dt.int16
BF16 = mybir.dt.bfloat16
AF = mybir.ActivationFunctionType

N_NODES = 25000
D_NODE = 128
D_EDGE_IN = 16
D_EH = 128
D_H = 256
D_DUAL = 128
AUG_W = 132  # 128 emb + 1 logit + 3 pad (528B rows)
P = 128
T = 512              # edges per macro tile (one fp32 PSUM bank)
CHUNKS = T // P      # 4 em chunks per macro
K_IDX = 16           # gathered rows per partition per indirect DMA
GROUP_E = P * K_IDX  # 2048 edges per gather group
MACROS_PER_GROUP = GROUP_E // T  # 4

N_CORES = 8
E_FULL = 400000
E_CORE = E_FULL // N_CORES       # 50000
G_FULL = 25                      # groups per core
E_PAD = G_FULL * GROUP_E         # 51200


def build_program(G, native_silu=True, repeats=1, fake_gather=False):
    """Build the per-core Bass program for G groups of 2048 edges.

    native_silu=False decomposes SiLU as x*sigmoid(x) (CoreSim has no Silu
    table); the math is identical.
    """
    nc = bacc.Bacc()
    ncols = G * K_IDX  # chunk-columns in the shuffled edge arrays

    aug = nc.declare_dram_parameter("aug_table", [N_NODES, AUG_W], BF16, isOutput=False)
    ea_sh = nc.declare_dram_parameter("ea_sh", [P, ncols, D_EDGE_IN], F32, isOutput=False)
    dual_sh = nc.declare_dram_parameter("dual_sh", [P, ncols, D_DUAL], F32, isOutput=False)
    # per-group gather indices, pre-wrapped host-side into the [16, n/16]
    # layout dma_gather expects (flat edge i at [i%16, i//16])
    src_sh = nc.declare_dram_parameter("src_sh", [P, ncols], I32, isOutput=False)
    dst_sh = nc.declare_dram_parameter("dst_sh", [P, ncols], I32, isOutput=False)

    w_in = {}
    for name, shape in [
        ("W_be1", [D_EDGE_IN, D_EH]), ("W_be2", [D_EH, D_EH]), ("W_be3", [D_EH, D_EH]),
        ("W_d1", [D_DUAL, D_H]), ("W_d2", [D_H, D_H]), ("W_do", [D_H, 1]),
        ("W_f1", [2 * D_NODE + D_EH + 3, D_H]), ("W_f2", [D_H, D_H]), ("W_bo", [D_H, 1]),
        ("b_be1", [D_EH]), ("g_be1", [D_EH]), ("bb_be1", [D_EH]),
        ("b_be2", [D_EH]), ("g_be2", [D_EH]), ("bb_be2", [D_EH]),
        ("b_be3", [D_EH]), ("g_be3", [D_EH]), ("bb_be3", [D_EH]),
        ("b_d1", [D_H]), ("b_d2", [D_H]), ("b_do", [1]),
        ("b_f1", [D_H]), ("b_f2", [D_H]), ("b_bo", [1]),
    ]:
        w_in[name] = nc.declare_dram_parameter(name, shape, F32, isOutput=False)

    out_d = nc.declare_dram_parameter("out", [G * GROUP_E], F32, isOutput=True)
    out_v = out_d.rearrange("(m t) -> m t", t=T)  # [G*4, T]

    with tile.TileContext(nc) as tc:
        with (
            tc.tile_pool(name="const", bufs=1) as const,
            tc.tile_pool(name="io", bufs=2) as io,
            tc.tile_pool(name="wst", bufs=1) as wst,
            tc.tile_pool(name="act", bufs=2) as act,
            tc.tile_pool(name="ps_mm", bufs=2, space="PSUM") as ps_mm,
            tc.tile_pool(name="ps_bc", bufs=2, space="PSUM") as ps_bc,
            tc.tile_pool(name="ps_tp", bufs=2, space="PSUM") as ps_tp,
            tc.tile_pool(name="ps_st", bufs=2, space="PSUM") as ps_st,
        ):
            # gpsimd library providing DMAGatherAnt
            nc.gpsimd.load_library(library_config.mlp)

            # ---- persistent constants ----
            ident = const.tile([P, P], F32)
            make_identity(nc, ident[:])
            ident_bf = const.tile([P, P], BF16)
            nc.vector.tensor_copy(ident_bf[:], ident[:])
            ones_f32 = const.tile([P, P], F32)
            nc.vector.memset(ones_f32[:], 1.0)
            ones_col = const.tile([P, 1], F32R)
            nc.vector.tensor_copy(ones_col[:], ones_f32[:, 0:1])
            ones_row = const.tile([1, P], F32R)
            nc.vector.tensor_copy(ones_row[:], ones_f32[0:1, :])

            def load_wr(name_, shape, src_ap):
                """DMA f32 weight then cast-copy to an f32r tile (fp32r matmul
                operands must be produced with fp32r rounding)."""
                stage = wst.tile(shape, F32, tag=f"wstage_{name_}")
                nc.sync.dma_start(stage[:], src_ap)
                wr = const.tile(shape, F32R, name=name_)
                nc.vector.tensor_copy(wr[:], stage[:])
                return wr

            wbe1 = load_wr("wbe1", [D_EDGE_IN, D_EH], w_in["W_be1"][:, :])
            wbe2 = load_wr("wbe2", [D_EH, D_EH], w_in["W_be2"][:, :])
            wbe3 = load_wr("wbe3", [D_EH, D_EH], w_in["W_be3"][:, :])
            wd1 = load_wr("wd1", [D_DUAL, D_H], w_in["W_d1"][:, :])
            wd2 = load_wr("wd2", [P, 2, D_H], w_in["W_d2"][:, :].rearrange("(ko ki) m -> ki ko m", ki=P))
            wdo = load_wr("wdo", [P, 2, 1], w_in["W_do"][:, :].rearrange("(ko ki) m -> ki ko m", ki=P))
            wf1 = load_wr("wf1", [P, 3, D_H], w_in["W_f1"][0:384, :].rearrange("(ko ki) m -> ki ko m", ki=P))
            wf1t = load_wr("wf1t", [3, D_H], w_in["W_f1"][384:387, :])
            bf1 = const.tile([P, 2], F32)
            nc.sync.dma_start(bf1[:], w_in["b_f1"][:].rearrange("(mc d) -> d mc", d=P))
            wf2 = load_wr("wf2", [P, 2, D_H], w_in["W_f2"][:, :].rearrange("(ko ki) m -> ki ko m", ki=P))
            wbo = load_wr("wbo", [P, 2, 1], w_in["W_bo"][:, :].rearrange("(ko ki) m -> ki ko m", ki=P))

            be_cols = {}
            for l in (1, 2, 3):
                for kind in ("b", "g", "bb"):
                    t_ = const.tile([P, 1], F32, name=f"{kind}_be{l}")
                    nc.sync.dma_start(t_[:], w_in[f"{kind}_be{l}"][:, None])
                    be_cols[(kind, l)] = t_
            bd1 = const.tile([P, 2], F32)
            nc.sync.dma_start(bd1[:], w_in["b_d1"][:].rearrange("(mc d) -> d mc", d=P))
            bd2 = const.tile([P, 2], F32)
            nc.sync.dma_start(bd2[:], w_in["b_d2"][:].rearrange("(mc d) -> d mc", d=P))
            bf2 = const.tile([P, 2], F32)
            nc.sync.dma_start(bf2[:], w_in["b_f2"][:].rearrange("(mc d) -> d mc", d=P))
            bdo = const.tile([1, 1], F32)
            nc.sync.dma_start(bdo[:], w_in["b_do"][None, :])
            bbo = const.tile([1, 1], F32)
            nc.sync.dma_start(bbo[:], w_in["b_bo"][None, :])

            def act_silu(out_ap, in_ap, bias=0.0, scale=1.0, tag="silu_tmp"):
                """out = silu(in*scale + bias); in may be PSUM."""
                if native_silu:
                    nc.scalar.activation(out_ap, in_ap, AF.Silu, bias=bias, scale=scale)
                    return
                z = act.tile(list(in_ap.shape), F32, tag="siluz")
                if isinstance(scale, float) and scale == 1.0 and isinstance(bias, float):
                    nc.vector.tensor_scalar_add(z[:], in_ap, bias)
                else:
                    nc.vector.tensor_scalar(
                        out=z[:], in0=in_ap, scalar1=scale, scalar2=bias,
                        op0=mybir.AluOpType.mult, op1=mybir.AluOpType.add)
                sg = act.tile(list(in_ap.shape), F32, tag="silus")
                nc.scalar.activation(sg[:], z[:], AF.Sigmoid)
                nc.vector.tensor_mul(out_ap, z[:], sg[:])

            def ln_silu(l, y_ps):
                """LayerNorm + affine + SiLU on one macro: y_ps [P,T] psum -> [P,T] sbuf."""
                y_sb = act.tile([P, T], F32R, tag="y")
                nc.vector.tensor_scalar_add(y_sb[:], y_ps[:], be_cols[("b", l)][:, 0:1])
                sq = act.tile([P, T], F32R, tag="sq")
                nc.vector.tensor_mul(sq[:], y_sb[:], y_sb[:])
                s1 = ps_st.tile([1, T], F32, tag="st")
                s2 = ps_st.tile([1, T], F32, tag="st")
                nc.tensor.matmul(s1[:], (ones_col[:]), (y_sb[:]), start=True, stop=True)
                nc.tensor.matmul(s2[:], (ones_col[:]), (sq[:]), start=True, stop=True)
                mu = act.tile([1, T], F32R, tag="mu")
                t1 = act.tile([1, T], F32R, tag="t1")
                t2 = act.tile([1, T], F32, tag="t2")
                nc.vector.tensor_scalar_mul(mu[:], s1[:], 1.0 / D_EH)
                nc.vector.tensor_scalar(
                    out=t1[:], in0=s2[:], scalar1=1.0 / D_EH, scalar2=1e-5,
                    op0=mybir.AluOpType.mult, op1=mybir.AluOpType.add)
                nc.vector.tensor_mul(t2[:], mu[:], mu[:])
                nc.vector.tensor_sub(t1[:], t1[:], t2[:])     # var + eps
                nc.scalar.sqrt(t2[:], t1[:])                  # std
                with nc.allow_low_precision(reason="fp32r rstd (4-byte, rounded mantissa)"):
                    nc.vector.reciprocal(t1[:], t2[:])        # rstd
                mu_bc = ps_bc.tile([P, T], F32, tag="bc")
                rs_bc = ps_bc.tile([P, T], F32, tag="bc")
                nc.tensor.matmul(mu_bc[:], (ones_row[:]), (mu[:]), start=True, stop=True)
                nc.tensor.matmul(rs_bc[:], (ones_row[:]), (t1[:]), start=True, stop=True)
                tn = act.tile([P, T], F32, tag="tnorm")
                nc.vector.tensor_sub(tn[:], y_sb[:], mu_bc[:])
                nc.vector.tensor_mul(tn[:], tn[:], rs_bc[:])
                x_out = act.tile([P, T], F32R, tag="x3" if l == 3 else "x")
                act_silu(x_out[:], tn[:], bias=be_cols[("bb", l)][:, 0:1],
                         scale=be_cols[("g", l)][:, 0:1], tag="ln_silu")
                return x_out

            def group_body(g):
                cg = g * K_IDX

                idx_s = io.tile([P, K_IDX], I32, tag="idx_s")
                idx_d = io.tile([P, K_IDX], I32, tag="idx_d")
                nc.sync.dma_start(idx_s[:], src_sh[:, cg:cg + K_IDX])
                nc.sync.dma_start(idx_d[:], dst_sh[:, cg:cg + K_IDX])
                ea_t = io.tile([P, K_IDX, D_EDGE_IN], F32, tag="ea_t")
                nc.sync.dma_start(ea_t[:], ea_sh[:, cg:cg + K_IDX, :])
                dual_t = io.tile([P, K_IDX, D_DUAL], F32, tag="dual_t")
                nc.sync.dma_start(dual_t[:], dual_sh[:, cg:cg + K_IDX, :])

                for mi in range(MACROS_PER_GROUP):
                    # ---- per-chunk row gathers ([P,1] offsets; 128 rows/instr) ----
                    src_gs = []
                    dst_gs = []
                    for cl in range(CHUNKS):
                        c = mi * CHUNKS + cl
                        sg_ = io.tile([P, AUG_W], BF16, tag="src_g", bufs=6)
                        dg_ = io.tile([P, AUG_W], BF16, tag="dst_g", bufs=6)
                        if fake_gather:
                            nc.sync.dma_start(sg_[:], aug[0:P, :])
                            nc.sync.dma_start(dg_[:], aug[0:P, :])
                        else:
                            nc.gpsimd.indirect_dma_start(
                                out=sg_[:], out_offset=None, in_=aug[:, :],
                                in_offset=bass.IndirectOffsetOnAxis(ap=idx_s[:, c:c + 1], axis=0))
                            nc.gpsimd.indirect_dma_start(
                                out=dg_[:], out_offset=None, in_=aug[:, :],
                                in_offset=bass.IndirectOffsetOnAxis(ap=idx_d[:, c:c + 1], axis=0))
                        src_gs.append(sg_)
                        dst_gs.append(dg_)

                    # ---- transposes to feature-major ----
                    pt = ps_tp.tile([D_EDGE_IN, T], F32, tag="tp")
                    for cl in range(CHUNKS):
                        c = mi * CHUNKS + cl
                        nc.tensor.transpose(pt[:, cl * P:(cl + 1) * P], ea_t[:, c, :], ident[:])
                    ea_fm = act.tile([D_EDGE_IN, T], F32R, tag="ea_fm")
                    nc.vector.tensor_copy(ea_fm[:], pt[:])

                    pt = ps_tp.tile([4, T], BF16, tag="tp")
                    for cl in range(CHUNKS):
                        nc.tensor.transpose(pt[:, cl * P:(cl + 1) * P],
                                            src_gs[cl][:, 128:132], ident_bf[:])
                    ls = act.tile([1, T], F32, tag="ls")
                    nc.vector.tensor_copy(ls[:], pt[0:1, :])

                    pt = ps_tp.tile([4, T], BF16, tag="tp")
                    for cl in range(CHUNKS):
                        nc.tensor.transpose(pt[:, cl * P:(cl + 1) * P],
                                            dst_gs[cl][:, 128:132], ident_bf[:])
                    ld = act.tile([1, T], F32, tag="ld")
                    nc.vector.tensor_copy(ld[:], pt[0:1, :])

                    fms = {}
                    for name in ("s", "d", "u"):
                        pt = ps_tp.tile([P, T], BF16 if name in ("s", "d") else F32, tag="tp")
                        for cl in range(CHUNKS):
                            if name == "s":
                                in_ap = src_gs[cl][:, 0:128]
                            elif name == "d":
                                in_ap = dst_gs[cl][:, 0:128]
                            else:
                                in_ap = dual_t[:, mi * CHUNKS + cl, :]
                            nc.tensor.transpose(pt[:, cl * P:(cl + 1) * P], in_ap,
                                                ident_bf[:] if name in ("s", "d") else ident[:])
                        fm = act.tile([P, T], F32R, tag=f"fm_{name}")
                        nc.vector.tensor_copy(fm[:], pt[:])
                        fms[name] = fm

                    # ---- BondEmbedding chain ----
                    yp = ps_mm.tile([P, T], F32, tag="mm")
                    nc.tensor.matmul(yp[:], (wbe1[:]), (ea_fm[:]), start=True, stop=True)
                    x1 = ln_silu(1, yp)
                    yp = ps_mm.tile([P, T], F32, tag="mm")
                    nc.tensor.matmul(yp[:], (wbe2[:]), (x1[:]), start=True, stop=True)
                    x2 = ln_silu(2, yp)
                    yp = ps_mm.tile([P, T], F32, tag="mm")
                    nc.tensor.matmul(yp[:], (wbe3[:]), (x2[:]), start=True, stop=True)
                    x3 = ln_silu(3, yp)

                    # ---- dual chain ----
                    d1_sb = act.tile([P, 2, T], F32R, tag="d1")
                    for mc in range(2):
                        dp = ps_mm.tile([P, T], F32, tag="mm")
                        nc.tensor.matmul(dp[:], (wd1[:, mc * P:(mc + 1) * P]),
                                         (fms["u"][:]), start=True, stop=True)
                        act_silu(d1_sb[:, mc, :], dp[:], bias=bd1[:, mc:mc + 1], tag="d1s")
                    d2_sb = act.tile([P, 2, T], F32R, tag="d2")
                    for mc in range(2):
                        dp = ps_mm.tile([P, T], F32, tag="mm")
                        for ki in range(2):
                            nc.tensor.matmul(dp[:], (wd2[:, ki, mc * P:(mc + 1) * P]),
                                             (d1_sb[:, ki, :]),
                                             start=(ki == 0), stop=(ki == 1))
                        act_silu(d2_sb[:, mc, :], dp[:], bias=bd2[:, mc:mc + 1], tag="d2s")
                    dpp = ps_st.tile([1, T], F32, tag="st")
                    for ki in range(2):
                        nc.tensor.matmul(dpp[:], (wdo[:, ki, :]), (d2_sb[:, ki, :]),
                                         start=(ki == 0), stop=(ki == 1))
                    tail = act.tile([3, T], F32R, tag="tail")
                    nc.scalar.activation(tail[0:1, :], dpp[:], AF.Sigmoid,
                                         bias=bdo[:, 0:1], scale=1.0)
                    ps_sig = act.tile([1, T], F32R, tag="ps_sig")
                    pd_sig = act.tile([1, T], F32R, tag="pd_sig")
                    nc.scalar.activation(ps_sig[:], ls[:], AF.Sigmoid)
                    nc.scalar.activation(pd_sig[:], ld[:], AF.Sigmoid)
                    nc.sync.dma_start(tail[1:2, :], ps_sig[:])
                    nc.sync.dma_start(tail[2:3, :], pd_sig[:])

                    # ---- main head ----
                    f1_sb = act.tile([P, 2, T], F32R, tag="f1")
                    for mc in range(2):
                        fp = ps_mm.tile([P, T], F32, tag="mm")
                        msl = slice(mc * P, (mc + 1) * P)
                        nc.tensor.matmul(fp[:], (wf1[:, 0, msl]), (fms["s"][:]),
                                         start=True, stop=False)
                        nc.tensor.matmul(fp[:], (wf1[:, 1, msl]), (fms["d"][:]),
                                         start=False, stop=False)
                        nc.tensor.matmul(fp[:], (wf1[:, 2, msl]), (x3[:]),
                                         start=False, stop=False)
                        nc.tensor.matmul(fp[:], (wf1t[:, msl]), (tail[:]),
                                         start=False, stop=True)
                        act_silu(f1_sb[:, mc, :], fp[:], bias=bf1[:, mc:mc + 1], tag="f1s")
                    f2_sb = act.tile([P, 2, T], F32R, tag="f2")
                    for mc in range(2):
                        fp = ps_mm.tile([P, T], F32, tag="mm")
                        for ki in range(2):
                            nc.tensor.matmul(fp[:], (wf2[:, ki, mc * P:(mc + 1) * P]),
                                             (f1_sb[:, ki, :]),
                                             start=(ki == 0), stop=(ki == 1))
                        act_silu(f2_sb[:, mc, :], fp[:], bias=bf2[:, mc:mc + 1], tag="f2s")
                    op = ps_st.tile([1, T], F32, tag="st")
                    for ki in range(2):
                        nc.tensor.matmul(op[:], (wbo[:, ki, :]), (f2_sb[:, ki, :]),
                                         start=(ki == 0), stop=(ki == 1))
                    o_sb = act.tile([1, T], F32, tag="o_sb")
                    nc.scalar.activation(o_sb[:], op[:], AF.Identity,
                                         bias=bbo[:, 0:1], scale=1.0)
                    nc.sync.dma_start(out_v[g * MACROS_PER_GROUP + mi:
                                            g * MACROS_PER_GROUP + mi + 1, :], o_sb[:])

            if repeats == 1:
                for g in range(G):
                    group_body(g)
            else:
                with tc.For_i(0, repeats, 1) as _rep:
                    for g in range(G):
                        group_body(g)

    return nc


def prep_edge_arrays(src, dst, ea, dual, e_pad):
    """Shuffle edge arrays into the device layouts (pure data movement).

    src/dst: int arrays [e]; ea: [e,16]; dual: [e,128]; zero-pads to e_pad.
    """
    e = len(src)
    pad = e_pad - e
    src = np.concatenate([np.asarray(src, np.int64), np.zeros(pad, np.int64)])
    dst = np.concatenate([np.asarray(dst, np.int64), np.zeros(pad, np.int64)])
    ea = np.concatenate([np.asarray(ea, np.float32), np.zeros((pad, D_EDGE_IN), np.float32)])
    dual = np.concatenate([np.asarray(dual, np.float32), np.zeros((pad, D_DUAL), np.float32)])

    def shuf2(x):
        return np.ascontiguousarray(x.reshape(e_pad // P, P, -1).transpose(1, 0, 2))

    def shufi(x):
        return np.ascontiguousarray(x.astype(np.int32).reshape(e_pad // P, P).T)

    return {
        "ea_sh": shuf2(ea),
        "dual_sh": shuf2(dual),
        "src_sh": shufi(src),
        "dst_sh": shufi(dst),
    }


def _prep_core_inputs(inputs, core, e_pad):
    e0 = core * E_CORE
    sl = slice(e0, e0 + E_CORE)
    return prep_edge_arrays(
        np.asarray(inputs["edge_index"][0][sl]),
        np.asarray(inputs["edge_index"][1][sl]),
        np.asarray(inputs["edge_attr"][sl]),
        np.asarray(inputs["dual_node_emb"][sl]), e_pad)


WEIGHT_NAMES = (
    "W_be1", "b_be1", "g_be1", "bb_be1", "W_be2", "b_be2", "g_be2", "bb_be2",
    "W_be3", "b_be3", "g_be3", "bb_be3", "W_d1", "b_d1", "W_d2", "b_d2",
    "W_do", "b_do", "W_f1", "b_f1", "W_f2", "b_f2", "W_bo", "b_bo")


def make_common_inputs(inputs):
    node_emb = np.asarray(inputs["node_embedding"], dtype=np.float32)
    logits = np.asarray(inputs["atom_reactivity_logits"], dtype=np.float32)
    import ml_dtypes
    aug = np.ascontiguousarray(np.concatenate(
        [node_emb, logits[:, None], np.zeros((N_NODES, 3), np.float32)],
        axis=1).astype(ml_dtypes.bfloat16))
    common = {"aug_table": aug}
    for name in WEIGHT_NAMES:
        common[name] = np.asarray(inputs[name], dtype=np.float32)
    return common


def kernel(**inputs):
    from concourse.bass_utils import run_bass_kernel_spmd

    nc = build_program(G_FULL)
    nc.finalize()
    common = make_common_inputs(inputs)
    in_maps = []
    for core in range(N_CORES):
        m = dict(common)
        m.update(_prep_core_inputs(inputs, core, E_PAD))
        in_maps.append(m)

    res = run_bass_kernel_spmd(nc, in_maps, list(range(N_CORES)))
    outs = [np.asarray(r["out"])[:E_CORE] for r in res.results]
    return np.concatenate(outs).astype(np.float32)

